# revision 19
# baseline (speedup 1.0000x reference)
"""Trainium2 Bass kernel for a dense transformer block (nn_Block_74500502716421).

Reference computation (per batch item, T=128 tokens, E=512 embed, H=8 heads,
D=64 head dim, F=2048 ffn hidden):

    h  = LN1(x);  q,k,v = per-head projections of h
    scores = causal_softmax(q k^T / sqrt(D));  o = concat_h(scores @ v)
    x2 = x + o @ proj_w + proj_b
    out = x2 + relu(LN2(x2) @ ff1_w + ff1_b) @ ff2_w + ff2_b

Distribution: pure data parallel — batch 512 split as 64 per NeuronCore
across 8 cores, all ~3.7M params replicated. No collectives.

Per-core structure (64 batch items, processed in 16 groups of 4 = 512
tokens):
  - Big matmuls (qkv/proj/ff1) in fp32r (full PE rate at moving dim 512),
    ff2 in bf16; contraction dim on partitions; activations kept
    feature-major (h^T, o^T, relu^T) via PE transposes.
  - LayerNorm stats token-major (bn_stats); rsqrt via bit-hack + Newton on
    DVE so ACT runs only the exp/relu/copy LUT set (no table reloads).
  - Attention: scores for 4 heads land in one PSUM bank (row-packed K=64
    matmul pairs, bf16) so one ACT exp covers 4 heads; causal mask +
    softmax denominators via one fused DVE op per head (bf16 in SBUF);
    probs normalized on GPSIMD; PE-transposed; col-packed AV pairs.
  - Emission is software-pipelined: LN1+qkv of group g are emitted before
    attention..ffn of group g-1, so every engine's in-order stream
    interleaves two groups and PE stays fed during attention latency.

LN gains/biases are folded exactly (float64 host math) into adjacent
projection weights, so the kernel applies pure (x - mu) * rstd.
"""

import sys

sys.path.insert(0, "/opt/trn_rl_repo")

from contextlib import ExitStack

import numpy as np

import concourse.bass as bass
import concourse.mybir as mybir
import concourse.tile as tile
from concourse import bacc
from concourse.bass import ts
from concourse.bass_utils import run_bass_kernel_spmd

N_CORES = 8
B_TOTAL = 512
B_CORE = B_TOTAL // N_CORES  # 64
T = 128
E = 512
H = 8
D = 64
F = 2048
GROUP = 4
N_GROUPS = B_CORE // GROUP  # 16

f32 = mybir.dt.float32
f32r = mybir.dt.float32r
bf16 = mybir.dt.bfloat16
i32 = mybir.dt.int32

RSQRT_MAGIC = 0x5F3759DF

_cache = {}


def _phase_front(nc, tc, P, g, dram, C, uniq=[0]):
    uniq[0] += 1
    g = g if True else g
    """Load x, LN1, transpose h^T, qkv projections for group g."""
    AF = mybir.ActivationFunctionType
    OP = mybir.AluOpType
    x_d = dram["x"]

    x_tm = []
    for b in range(GROUP):
        bg = g * GROUP + b
        xt = P["x_tm"].tile([T, E], f32, tag="x_tm", bufs=8, name=f"x_{bg}")
        nc.sync.dma_start(out=xt, in_=x_d[bg])
        x_tm.append(xt)

    def rsqrt_newton(y, veps, name):
        yi = y.bitcast(i32)
        nc.vector.tensor_single_scalar(
            out=yi, in_=veps.bitcast(i32), scalar=1, op=OP.arith_shift_right
        )
        nc.vector.tensor_scalar(
            out=yi, in0=yi, scalar1=-1, scalar2=RSQRT_MAGIC, op0=OP.mult, op1=OP.add
        )
        tmp = P["small"].tile(list(y.shape), f32, tag="nt", bufs=4, name=f"nt_{name}")
        for _ in range(3):
            nc.vector.tensor_mul(out=tmp, in0=y, in1=y)
            nc.vector.tensor_mul(out=tmp, in0=tmp, in1=veps)
            nc.vector.tensor_scalar(
                out=tmp, in0=tmp, scalar1=-0.5, scalar2=1.5, op0=OP.mult, op1=OP.add
            )
            nc.vector.tensor_mul(out=y, in0=y, in1=tmp)

    def layernorm_batch(xts, name):
        mvs = []
        veps = P["small"].tile([T, GROUP], f32, tag="veps", bufs=3, name=f"ve_{name}")
        for b, xt in enumerate(xts):
            stats = P["small"].tile([T, 6], f32, tag="stats", name=f"st_{name}_{b}")
            nc.vector.bn_stats(out=stats, in_=xt)
            mv = P["small"].tile([T, 2], f32, tag="mv", bufs=10, name=f"mv_{name}_{b}")
            nc.vector.bn_aggr(out=mv, in_=stats)
            nc.gpsimd.tensor_scalar_add(
                out=veps[:, b : b + 1], in0=mv[:, 1:2], scalar1=1e-5
            )
            mvs.append(mv)
        rstd = P["small"].tile([T, GROUP], f32, tag="rstd", bufs=3, name=f"rs_{name}")
        rsqrt_newton(rstd, veps, name)
        hts = []
        for b, xt in enumerate(xts):
            ht = P["h_tm"].tile([T, E], f32r, tag="h_tm", name=f"h_{name}_{b}")
            nc.vector.tensor_scalar(
                out=ht, in0=xt, scalar1=mvs[b][:, 0:1], scalar2=rstd[:, b : b + 1],
                op0=OP.subtract, op1=OP.mult,
            )
            hts.append(ht)
        return hts

    h_T = P["hT"].tile([T, 4, E], f32r, tag="hT", bufs=2, name=f"hT_{g}")
    hts = layernorm_batch(x_tm, f"1_{g}")
    for b in range(GROUP):
        tpb = P["psA"].tile([T, 4, T], f32r, tag="mm_rot", name=f"tp1_{g}_{b}")
        for j in range(4):
            nc.tensor.transpose(tpb[:, j, :], hts[b][:, ts(j, T)], C["id_f32r"])
        nc.vector.tensor_copy(out=h_T[:, :, ts(b, T)], in_=tpb)

    q_T, k_T = [], []
    for proj, store in ((0, q_T), (1, k_T)):
        for mt in range(4):
            ps = P["psA"].tile([T, E], f32, tag="mm_rot", name=f"qk_{g}_{proj}_{mt}")
            for kt in range(4):
                nc.tensor.matmul(
                    ps,
                    C["wqkv"][:, proj, kt, ts(mt, T)],
                    h_T[:, kt, :],
                    start=(kt == 0),
                    stop=(kt == 3),
                )
            sb = P["qk"].tile(
                [T, E], bf16, tag=f"qk{proj}", bufs=8, name=f"qkT_{g}_{proj}_{mt}"
            )
            nc.scalar.add(out=sb, in_=ps, add=C["qkvb"][:, proj, mt : mt + 1])
            store.append(sb)

    v_sb = []
    for b in range(GROUP):
        ps = P["psA"].tile([T, E], f32, tag="mm_rot", name=f"vps_{g}_{b}")
        nc.tensor.matmul(ps, C["ones1"], C["vb_row"], start=True, stop=False)
        for kt in range(4):
            nc.tensor.matmul(
                ps, h_T[:, kt, ts(b, T)], C["wqkv"][:, 2, kt, :],
                start=False, stop=(kt == 3),
            )
        vb = P["v"].tile([T, E], bf16, tag="v", bufs=8, name=f"v_{g}_{b}")
        nc.scalar.copy(out=vb, in_=ps)
        v_sb.append(vb)

    return {"x_tm": x_tm, "q_T": q_T, "k_T": k_T, "v_sb": v_sb, "layernorm": layernorm_batch}


def _phase_back(nc, tc, P, g, dram, C, st):
    """Attention + proj + LN2 + ffn + store for group g (state from front)."""
    AF = mybir.ActivationFunctionType
    OP = mybir.AluOpType
    out_d = dram["out"]
    x_tm = st["x_tm"]
    q_T, k_T, v_sb = st["q_T"], st["k_T"], st["v_sb"]

    # ---- attention, stage-wise across the 4 batch items ----
    # stage 1: scores + exp. Even heads fill PSUM bank A, odd heads bank B:
    # each row-packed concurrent (even, odd) matmul pair writes two
    # different banks, and writes within a bank are sequential. em slot s
    # of half 0/1 holds head 2s / 2s+1.
    em_all, sums_all = [], []
    for b in range(GROUP):
        em = P["em"].tile([T, 2, 4, T], bf16, tag="em", bufs=4, name=f"em_{g}_{b}")
        scs = [
            P["psB"].tile([T, 4, T], f32, tag="att", name=f"sc_{g}_{b}_{par}")
            for par in range(2)
        ]
        for j in range(4):
            for par in range(2):
                h = 2 * j + par
                hslice = slice(64 * par, 64 * (par + 1))
                nc.tensor.matmul(
                    scs[par][:, j, :],
                    q_T[j][hslice, ts(b, T)],
                    k_T[j][hslice, ts(b, T)],
                    start=True, stop=True,
                )
        for par in range(2):
            nc.scalar.activation(
                out=em[:, par, :, :], in_=scs[par], func=AF.Exp, scale=0.125
            )
        em_all.append(em)
    # stage 2: causal mask + softmax denominators (fused, bf16 in SBUF)
    for b in range(GROUP):
        sums = P["small"].tile([T, H], f32, tag="sums", name=f"sums_{g}_{b}")
        for h in range(H):
            es = em_all[b][:, h % 2, h // 2, :]
            nc.vector.scalar_tensor_tensor(
                out=es, in0=es, scalar=1.0,
                in1=C["mask01"], op0=OP.mult, op1=OP.mult,
                accum_out=sums[:, h : h + 1],
            )
        sums_all.append(sums)
    # stage 3: normalize (GPSIMD) + PE transpose + drain
    at_all = []
    for b in range(GROUP):
        recips = P["small"].tile([T, H], f32, tag="recips", name=f"rec_{g}_{b}")
        nc.vector.reciprocal(out=recips, in_=sums_all[b])
        at_ps = P["psB"].tile([T, H, T], bf16, tag="att", name=f"atp_{g}_{b}")
        for h in range(H):
            es = em_all[b][:, h % 2, h // 2, :]
            nc.gpsimd.tensor_scalar_mul(
                out=es, in0=es, scalar1=recips[:, h : h + 1],
            )
            nc.tensor.transpose(at_ps[:, h, :], es, C["id_bf16"])
        at_sb = P["att_sb"].tile(
            [T, H, T], bf16, tag="attnT", bufs=3, name=f"aT_{g}_{b}"
        )
        nc.scalar.copy(out=at_sb, in_=at_ps)
        at_all.append(at_sb)
    # stage 4: AV (col-packed pairs) + drain to o^T
    o_T = P["oT"].tile([T, 4, E], f32r, tag="oT", bufs=2, name=f"oT_{g}")
    for b in range(GROUP):
        o_ps = P["psB"].tile([T, 4, T], f32, tag="att", name=f"o_{g}_{b}")
        for j in range(4):
            nc.tensor.matmul(
                o_ps[0:64, j, :], v_sb[b][:, ts(2 * j, D)], at_all[b][:, 2 * j, :],
                start=True, stop=True, tile_position=(0, 0),
            )
            nc.tensor.matmul(
                o_ps[64:128, j, :], v_sb[b][:, ts(2 * j + 1, D)],
                at_all[b][:, 2 * j + 1, :],
                start=True, stop=True, tile_position=(0, 64),
            )
        nc.vector.tensor_copy(out=o_T[:, :, ts(b, T)], in_=o_ps)

    # ---- proj + residual ----
    for b in range(GROUP):
        ps = P["psA"].tile([T, E], f32, tag="mm_rot", name=f"pj_{g}_{b}")
        nc.tensor.matmul(ps, C["ones1"], C["pb_row"], start=True, stop=False)
        for kt in range(4):
            nc.tensor.matmul(
                ps, o_T[:, kt, ts(b, T)], C["proj_w"][:, kt, :],
                start=False, stop=(kt == 3),
            )
        nc.vector.tensor_add(out=x_tm[b], in0=x_tm[b], in1=ps)  # x2 in place

    # ---- LN2 + transpose ----
    h2_T = P["hT"].tile([T, 4, E], f32r, tag="hT", bufs=2, name=f"h2T_{g}")
    hts2 = st["layernorm"](x_tm, f"2_{g}")
    for b in range(GROUP):
        tpb = P["psA"].tile([T, 4, T], f32r, tag="mm_rot", name=f"tp2_{g}_{b}")
        for j in range(4):
            nc.tensor.transpose(tpb[:, j, :], hts2[b][:, ts(j, T)], C["id_f32r"])
        nc.vector.tensor_copy(out=h2_T[:, :, ts(b, T)], in_=tpb)

    # ---- ffn ----
    r_all = []
    for mt in range(16):
        ps1 = P["psA"].tile([T, E], f32, tag="mm_rot", name=f"ff1_{g}_{mt}")
        for kt in range(4):
            nc.tensor.matmul(
                ps1,
                C["ff1_w"][:, kt, ts(mt, T)],
                h2_T[:, kt, :],
                start=(kt == 0),
                stop=(kt == 3),
            )
        r = P["relu"].tile([T, E], bf16, tag="relu", bufs=16, name=f"r_{g}_{mt}")
        nc.scalar.activation(
            out=r, in_=ps1, func=AF.Relu, bias=C["ff1b"][:, mt : mt + 1]
        )
        r_all.append(r)
    for half in range(2):
        accs = {}
        for b in (2 * half, 2 * half + 1):
            acc = P["psC"].tile([T, E], f32, tag="ff2acc", name=f"ff2_{g}_{b}")
            nc.tensor.matmul(acc, C["ones1"], C["ffb_row"], start=True, stop=False)
            accs[b] = acc
        for mt in range(16):
            for b in (2 * half, 2 * half + 1):
                nc.tensor.matmul(
                    accs[b],
                    r_all[mt][:, ts(b, T)],
                    C["ff2_w"][:, mt, :],
                    start=False,
                    stop=(mt == 15),
                )
        for b in (2 * half, 2 * half + 1):
            bg = g * GROUP + b
            nc.vector.tensor_add(out=x_tm[b], in0=x_tm[b], in1=accs[b])
            nc.sync.dma_start(out=out_d[bg], in_=x_tm[b])


def _build(n_groups=N_GROUPS, reps=1, hw_loop=0):
    nc = bacc.Bacc("TRN2", target_bir_lowering=False, debug=False)
    dram = {
        "x": nc.dram_tensor("x", [B_CORE, T, E], f32, kind="ExternalInput").ap(),
        "wqkv": nc.dram_tensor("wqkv", [3, 4, T, E], f32, kind="ExternalInput").ap(),
        "qkvb": nc.dram_tensor("qkvb", [3, E], f32, kind="ExternalInput").ap(),
        "proj_w": nc.dram_tensor("proj_w", [4, T, E], f32, kind="ExternalInput").ap(),
        "pb": nc.dram_tensor("pb", [1, E], f32, kind="ExternalInput").ap(),
        "ff1_w": nc.dram_tensor("ff1_w", [4, T, F], f32, kind="ExternalInput").ap(),
        "ff1b": nc.dram_tensor("ff1b", [F], f32, kind="ExternalInput").ap(),
        "ff2_w": nc.dram_tensor("ff2_w", [16, T, E], f32, kind="ExternalInput").ap(),
        "ffb": nc.dram_tensor("ffb", [1, E], f32, kind="ExternalInput").ap(),
        "ident": nc.dram_tensor("ident", [T, T], f32, kind="ExternalInput").ap(),
        "ident_bf16": nc.dram_tensor(
            "ident_bf16", [T, T], bf16, kind="ExternalInput"
        ).ap(),
        "mask01": nc.dram_tensor("mask01", [T, T], bf16, kind="ExternalInput").ap(),
        "out": nc.dram_tensor("out", [B_CORE, T, E], f32, kind="ExternalOutput").ap(),
    }

    with tile.TileContext(nc) as tc, ExitStack() as ctx:
        wpool = ctx.enter_context(tc.tile_pool(name="weights", bufs=1))
        consts = {}
        consts["wqkv"] = wpool.tile([T, 3, 4, E], f32r, name="w_qkv")
        consts["proj_w"] = wpool.tile([T, 4, E], f32r, name="w_proj")
        consts["ff1_w"] = wpool.tile([T, 4, F], f32r, name="w_ff1")
        consts["ff2_w"] = wpool.tile([T, 16, E], bf16, name="w_ff2")
        consts["qkvb"] = wpool.tile([T, 3, 4], f32, name="c_qkvb")
        consts["ff1b"] = wpool.tile([T, 16], f32, name="c_ff1b")
        consts["vb_row"] = wpool.tile([1, E], f32r, name="c_vb")
        consts["pb_row"] = wpool.tile([1, E], f32r, name="c_pb")
        consts["ffb_row"] = wpool.tile([1, E], f32r, name="c_ffb")
        consts["ones1"] = wpool.tile([1, T], f32r, name="c_ones")
        consts["id_f32r"] = wpool.tile([T, T], f32r, name="c_idf")
        consts["id_bf16"] = wpool.tile([T, T], bf16, name="c_idb")
        consts["mask01"] = wpool.tile([T, T], bf16, name="c_mask")

        nc.sync.dma_start(
            out=consts["qkvb"], in_=dram["qkvb"].rearrange("p (m q) -> q p m", q=T)
        )
        nc.sync.dma_start(
            out=consts["ff1b"], in_=dram["ff1b"].rearrange("(m q) -> q m", q=T)
        )

        nc.sync.dma_start(out=consts["id_bf16"], in_=dram["ident_bf16"])
        nc.sync.dma_start(out=consts["mask01"], in_=dram["mask01"])

        def round_load(dst, src_ap, shape):
            # stage through x_tm pool slots (same [T, E] f32 geometry)
            rows, cols = shape
            st = P["x_tm"].tile([T, E], f32, tag="x_tm", bufs=8, name="wstage")
            nc.sync.dma_start(out=st[:rows, :cols], in_=src_ap)
            nc.vector.tensor_copy(out=dst, in_=st[:rows, :cols])

        def load_front_weights():
            round_load(consts["id_f32r"], dram["ident"], [T, T])
            for p in range(3):
                for kt in range(4):
                    round_load(consts["wqkv"][:, p, kt, :], dram["wqkv"][p, kt], [T, E])
            round_load(consts["vb_row"], dram["qkvb"][2:3, :], [1, E])
            ones_st = P["x_tm"].tile([T, E], f32, tag="x_tm", bufs=8, name="onesstage")
            nc.vector.memset(ones_st[:1, :T], 1.0)
            nc.vector.tensor_copy(out=consts["ones1"], in_=ones_st[:1, :T])

        def load_back_weights():
            for kt in range(4):
                round_load(consts["proj_w"][:, kt, :], dram["proj_w"][kt], [T, E])
            round_load(consts["pb_row"], dram["pb"], [1, E])
            for kt in range(4):
                for c in range(4):
                    round_load(
                        consts["ff1_w"][:, kt, ts(c, E)],
                        dram["ff1_w"][kt][:, ts(c, E)],
                        [T, E],
                    )
            for mt in range(16):
                round_load(consts["ff2_w"][:, mt, :], dram["ff2_w"][mt], [T, E])
            round_load(consts["ffb_row"], dram["ffb"], [1, E])

        P = {}
        P["x_tm"] = ctx.enter_context(tc.tile_pool(name="x_tm", bufs=8))
        P["h_tm"] = ctx.enter_context(tc.tile_pool(name="h_tm", bufs=2))
        P["hT"] = ctx.enter_context(tc.tile_pool(name="hT", bufs=2))
        P["qk"] = ctx.enter_context(tc.tile_pool(name="qk", bufs=8))
        P["v"] = ctx.enter_context(tc.tile_pool(name="v", bufs=8))
        P["em"] = ctx.enter_context(tc.tile_pool(name="em", bufs=4))
        P["att_sb"] = ctx.enter_context(tc.tile_pool(name="att_sb", bufs=3))
        P["oT"] = ctx.enter_context(tc.tile_pool(name="oT", bufs=2))
        P["relu"] = ctx.enter_context(tc.tile_pool(name="relu", bufs=16))
        P["small"] = ctx.enter_context(tc.tile_pool(name="small", bufs=6))
        P["psA"] = ctx.enter_context(tc.tile_pool(name="psA", bufs=3, space="PSUM"))
        P["psB"] = ctx.enter_context(tc.tile_pool(name="psB", bufs=3, space="PSUM"))
        P["psC"] = ctx.enter_context(tc.tile_pool(name="psC", bufs=2, space="PSUM"))

        def emit_all(first=False):
            prev = None
            for g in range(n_groups):
                if first and g == 0:
                    load_front_weights()
                st = _phase_front(nc, tc, P, g, dram, consts)
                if first and g == 0:
                    load_back_weights()
                if prev is not None:
                    _phase_back(nc, tc, P, prev[0], dram, consts, prev[1])
                prev = (g, st)
            _phase_back(nc, tc, P, prev[0], dram, consts, prev[1])

        if hw_loop:
            load_front_weights()
            load_back_weights()
            with tc.For_i(0, hw_loop, 1):
                emit_all()
        else:
            for rep in range(reps):
                emit_all(first=(rep == 0))

    nc.compile()
    return nc


def _prep_weights(inputs):
    """Host-side exact folding + reshaping of weights (float64 math)."""
    gets = {k: np.asarray(inputs[k], dtype=np.float64) for k in inputs}
    g1, b1 = gets["ln1_g"], gets["ln1_b"]
    g2, b2 = gets["ln2_g"], gets["ln2_b"]

    wqkv = np.empty((3, 4, T, E), np.float32)
    qkvb = np.empty((3, E), np.float32)
    for i, wname in enumerate(("wq", "wk", "wv")):
        w = gets[wname]  # [H, E, D]
        wf = w * g1[None, :, None]
        bias = np.einsum("e,hed->hd", b1, w).reshape(E)
        wr = wf.transpose(1, 0, 2).reshape(E, H * D)
        wqkv[i] = wr.reshape(4, T, H * D).astype(np.float32)
        qkvb[i] = bias.astype(np.float32)

    proj_w = gets["proj_w"].reshape(4, T, E).astype(np.float32)
    pb = gets["proj_b"].reshape(1, E).astype(np.float32)

    ff1 = gets["ff1_w"] * g2[:, None]
    ff1b = (gets["ff1_b"] + b2 @ gets["ff1_w"]).astype(np.float32)
    ff1_w = ff1.reshape(4, T, F).astype(np.float32)
    ff2_w = gets["ff2_w"].reshape(16, T, E).astype(np.float32)
    ffb = gets["ff2_b"].reshape(1, E).astype(np.float32)

    import ml_dtypes

    tt, ss = np.meshgrid(np.arange(T), np.arange(T), indexing="ij")
    mask01 = (ss <= tt).astype(np.float32).astype(ml_dtypes.bfloat16)

    return {
        "wqkv": wqkv,
        "qkvb": qkvb,
        "proj_w": proj_w,
        "pb": pb,
        "ff1_w": ff1_w,
        "ff1b": ff1b,
        "ff2_w": ff2_w,
        "ffb": ffb,
        "ident": np.eye(T, dtype=np.float32),
        "ident_bf16": np.eye(T, dtype=np.float32).astype(ml_dtypes.bfloat16),
        "mask01": mask01,
    }


def kernel(**inputs) -> np.ndarray:
    x = np.asarray(inputs["x"], dtype=np.float32)
    weights = _prep_weights(inputs)

    if "nc" not in _cache:
        _cache["nc"] = _build()
    nc = _cache["nc"]

    in_maps = []
    for c in range(N_CORES):
        m = dict(weights)
        m["x"] = np.ascontiguousarray(x[c * B_CORE : (c + 1) * B_CORE])
        in_maps.append(m)
    res = run_bass_kernel_spmd(nc, in_maps, core_ids=list(range(N_CORES)))
    out = np.concatenate([res.results[c]["out"] for c in range(N_CORES)], axis=0)
    return out
